# revision 24
# baseline (speedup 1.0000x reference)
"""Self-contained Trainium2 Bass kernel for nn_CA_9363028705415 (sparse_attention).

Computes, per batch b:
    Q = relu(x[b] @ qW1 + qb1) @ qW2 + qb2          # [M, K]
    Kt = relu(x[b] @ kW1 + kb1) @ kW2 + kb2         # [M, K]
    S = Q @ Kt.T                                    # [M, M]
    out[b] = softmax(S / rowmax(S), axis=-1)        # max-DIVISION normalization

Shapes: B=16, M=2048, D=128, H=256, K=64.  Output [16, 2048, 2048] f32 (256 MB).

Sharding: data-parallel over batch across 8 NeuronCores; 2 batches/core; tiny
MLP weights replicated.  Single NEFF run SPMD via run_bass_kernel_spmd.

Design notes (HW-calibrated via perfetto traces):
  - Output written to DRAM as fp16, upcast to f32 on the host: halves the
    dominant HBM write traffic (rows sum to 1, fp16 rel err ~5e-4).
  - x loaded pre-cast f32->bf16 by gpsimd (SWDGE) DMA.
  - Per row-tile pipeline:
      PE   S = Q K^T -> f32 PSUM (TRN2: matmul PSUM output must be f32).
           Matmuls are emitted one iteration ahead so the next tile's PSUM
           slot is claimed before any interleaved chunk's: S matmuls never
           queue behind a chunk's ACT evacuation.
      DVE  fused PSUM->SBUF fp16 evac + row-max accum (~2.1us, frees PSUM)
      DVE  1/max reciprocal -- separate from the 1/sum reciprocal so exp
           never waits on the previous tile's accumulator-read round trip
      ACT  exp(S * (1/max)) fp16->fp16 + fused row-sum accum (~2.25us)
      DVE  1/sum reciprocal + normalize t * (1/sum) fp16->fp16 at DVE 4x
           mode (~0.7us)
      DMA  1 MB fp16 output chunks (2 row-tiles per DMA)
  - DVE is the binding engine in steady state (evac 2.1 + norm 0.7 +
    recips 0.15 ~= 3.0us/tile vs ACT 2.25); measured cadence is 3.00us.
  - Batch-1 phase A (transposes + MLP) interleaves into batch-0's S loop at
    iterations 1..8 (early enough that the batch boundary has no bubble);
    MLP evacs ride ACT (exp/identity/relu/copy share one activation table
    -> no table reloads).
"""

import numpy as np
import ml_dtypes

import concourse.bass as bass
import concourse.mybir as mybir
from concourse import bacc
import concourse.tile as tile
from concourse.bass import ts
from concourse.bass_utils import run_bass_kernel_spmd

F32 = mybir.dt.float32
BF16 = mybir.dt.bfloat16
FP16 = mybir.dt.float16
AF = mybir.ActivationFunctionType
ALU = mybir.AluOpType

N_CORES = 8
B, M, D, H, KF = 16, 2048, 128, 256, 64
BPC = B // N_CORES     # batches per core
MT = M // 128          # 16 row-tiles per batch
FC = M // 512          # 4 matmul free-chunks of 512
PAIR = 2               # row-tiles per output DMA (1 MB fp16 chunks)

RELU_ENGINES = ("act", "act", "act", "act")  # batch-1 MLP1 evac engines
QKT_EVAC = ("act", "act")                    # batch-1 MLP2 evac engines


def _evac_bias(nc, engine, out, in_, bias, relu):
    """out = [relu](in_ + bias), bias is [P,1] per-partition AP."""
    if engine == "act":
        nc.scalar.activation(
            out, in_, AF.Relu if relu else AF.Identity, bias=bias, scale=1.0
        )
    else:
        if relu:
            nc.vector.tensor_scalar(out, in_, bias, 0.0, op0=ALU.add, op1=ALU.max)
        else:
            nc.vector.tensor_scalar(out, in_, bias, None, op0=ALU.add)


def build_nc():
    nc = bacc.Bacc()

    x = nc.dram_tensor("x", [BPC, M, D], F32, kind="ExternalInput")
    w1d, b1d, w2d, b2d = {}, {}, {}, {}
    for h in ("q", "k"):
        w1d[h] = nc.dram_tensor(f"{h}W1", [D, H], F32, kind="ExternalInput")
        b1d[h] = nc.dram_tensor(f"{h}b1", [H], F32, kind="ExternalInput")
        w2d[h] = nc.dram_tensor(f"{h}W2", [H, KF], F32, kind="ExternalInput")
        b2d[h] = nc.dram_tensor(f"{h}b2", [KF], F32, kind="ExternalInput")
    out = nc.dram_tensor("out", [BPC, M, M], FP16, kind="ExternalOutput")

    ident_np = np.eye(128, dtype=ml_dtypes.bfloat16)
    ident_dram = nc.inline_tensor(ident_np, name="ident_data")

    # [b, p, n, d]: token (n*128+p), feature d
    x_r = x[:].rearrange("b (n p) d -> b p n d", p=128)
    # [b, p, n, m]: out[b, n*128+p, m]
    out_r = out[:].rearrange("b (n p) m -> b p n m", p=128)

    with tile.TileContext(nc) as tc:
        with (
            tc.tile_pool(name="consts", bufs=1) as consts,
            tc.tile_pool(name="xin", bufs=2) as xin_pool,
            tc.tile_pool(name="xt", bufs=2) as xt_pool,
            tc.tile_pool(name="ht", bufs=2) as ht_pool,
            tc.tile_pool(name="qkt", bufs=2) as qkt_pool,
            tc.tile_pool(name="sc", bufs=3) as sc_pool,
            tc.tile_pool(name="texp", bufs=3) as t_pool,
            tc.tile_pool(name="osb", bufs=3) as out_pool,
            tc.tile_pool(name="small", bufs=6) as small_pool,
            tc.tile_pool(name="psum", bufs=2, space="PSUM") as psum_pool,
        ):
            # ---- x loads for batch 0 first (gpsimd DMA casts f32->bf16) ----
            ident = consts.tile([128, 128], BF16, tag="ident")
            nc.sync.dma_start(out=ident, in_=ident_dram[:])
            xf = {}
            for b in range(BPC):
                xf[b] = xin_pool.tile([128, MT, 128], BF16, tag=f"xf{b}", name="xf")
            for g in range(4):
                nc.gpsimd.dma_start(
                    out=xf[0][:, g * 4 : (g + 1) * 4, :],
                    in_=x_r[0][:, g * 4 : (g + 1) * 4, :],
                )

            # ---- PE p-state warmup: dense dummy transposes on the identity
            # while waiting for x (cold PE runs matmuls 2-2.5x slower; this
            # burst runs in the 8-11us x-DMA window and lifts the clock
            # before the real transposes arrive) ----
            warm = psum_pool.tile([128, 1024], BF16, tag="ps", name="warm")
            for i in range(24):
                nc.tensor.transpose(warm[:, ts(i % 8, 128)], ident, ident)

            # ---- constants (weights cast f32->bf16 by gpsimd DMA) ----
            w1, w2, b1, b2 = {}, {}, {}, {}
            for h in ("q", "k"):
                w1[h] = consts.tile([D, H], BF16, tag=f"w1{h}", name=f"w1{h}")
                nc.gpsimd.dma_start(out=w1[h], in_=w1d[h][:])  # cast f32->bf16
                w2[h] = consts.tile([128, 2, KF], BF16, tag=f"w2{h}", name=f"w2{h}")
                nc.gpsimd.dma_start(
                    out=w2[h], in_=w2d[h][:].rearrange("(c p) k -> p c k", p=128)
                )
                b1[h] = consts.tile([128, 2], F32, tag=f"b1{h}", name=f"b1{h}")
                nc.sync.dma_start(
                    out=b1[h], in_=b1d[h][:].rearrange("(c p) -> p c", p=128)
                )
                b2[h] = consts.tile([KF, 1], F32, tag=f"b2{h}", name=f"b2{h}")
                nc.sync.dma_start(
                    out=b2[h], in_=b2d[h][:].rearrange("(k o) -> k o", o=1)
                )
            for g in range(4):
                nc.gpsimd.dma_start(
                    out=xf[1][:, g * 4 : (g + 1) * 4, :],
                    in_=x_r[1][:, g * 4 : (g + 1) * 4, :],
                )

            def phase_a_chunks(b, fast=False):
                """Phase-A emission chunks for batch b.  fast=True (batch 0)
                splits evacs across both engines to shorten the serial ramp;
                otherwise engines follow the tuned patterns."""
                ctx = {}
                ctx["xT"] = xt_pool.tile([128, M], BF16, tag="xt", name="xT")

                def c_tp(g):
                    def go():
                        tp = psum_pool.tile([128, 1024], BF16, tag="ps", name="tp")
                        for it in range(8):
                            nc.tensor.transpose(
                                tp[:, ts(it, 128)], xf[b][:, g * 8 + it, :], ident
                            )
                        if fast:
                            nc.scalar.copy(
                                ctx["xT"][:, g * 1024 : g * 1024 + 512], tp[:, 0:512]
                            )
                            nc.vector.tensor_scalar(
                                ctx["xT"][:, g * 1024 + 512 : (g + 1) * 1024],
                                tp[:, 512:1024], 0.0, None, op0=ALU.add,
                            )
                        else:
                            nc.vector.tensor_scalar(
                                ctx["xT"][:, ts(g, 1024)], tp, 0.0, None, op0=ALU.add
                            )
                    return go

                def c_mlp1(h, pc, eng):
                    def go():
                        if ("ht", h) not in ctx:
                            ctx[("ht", h)] = ht_pool.tile(
                                [128, 2, M], BF16, tag=f"ht{h}", name=f"ht{h}"
                            )
                        ps1 = psum_pool.tile([128, M], F32, tag="ps", name="ps1")
                        for fc in range(FC):
                            nc.tensor.matmul(
                                ps1[:, ts(fc, 512)],
                                lhsT=w1[h][:, ts(pc, 128)],
                                rhs=ctx["xT"][:, ts(fc, 512)],
                                start=True,
                                stop=True,
                            )
                        if fast:
                            for e, hf in (("act", 0), ("dve", 1)):
                                _evac_bias(
                                    nc, e,
                                    ctx[("ht", h)][:, pc, ts(hf, 1024)],
                                    ps1[:, ts(hf, 1024)],
                                    b1[h][:, pc : pc + 1],
                                    relu=True,
                                )
                        else:
                            _evac_bias(
                                nc, eng,
                                ctx[("ht", h)][:, pc, :],
                                ps1,
                                b1[h][:, pc : pc + 1],
                                relu=True,
                            )
                    return go

                def c_mlp2(h, eng):
                    def go():
                        ps2 = psum_pool.tile([KF, M], F32, tag="ps", name="ps2")
                        for fc in range(FC):
                            for kc in range(2):
                                nc.tensor.matmul(
                                    ps2[:, ts(fc, 512)],
                                    lhsT=w2[h][:, kc, :],
                                    rhs=ctx[("ht", h)][:, kc, ts(fc, 512)],
                                    start=(kc == 0),
                                    stop=(kc == 1),
                                )
                        q = qkt_pool.tile([KF, M], BF16, tag=f"qkt{h}", name=f"qkt{h}")
                        ctx[("qkt", h)] = q
                        if fast:
                            # chunked evac so the first S matmuls can start on
                            # column chunk 0 before the rest are evacuated
                            for fc in range(FC):
                                _evac_bias(
                                    nc, ("act", "dve")[fc % 2],
                                    q[:, ts(fc, 512)],
                                    ps2[:, ts(fc, 512)],
                                    b2[h],
                                    relu=False,
                                )
                        else:
                            _evac_bias(nc, eng, q, ps2, b2[h], relu=False)
                    return go

                chunks = [c_tp(0), c_tp(1)]
                for i, (h, pc) in enumerate(
                    [("q", 0), ("k", 0), ("q", 1), ("k", 1)]
                ):
                    chunks.append(c_mlp1(h, pc, RELU_ENGINES[i]))
                chunks.append(c_mlp2("q", QKT_EVAC[0]))
                chunks.append(c_mlp2("k", QKT_EVAC[1]))
                return ctx, chunks

            def s_loop(b, qkt, next_chunks):
                """Emit the S+softmax loop for batch b.  The 1/max and 1/sum
                reciprocals are separate instructions so exp(rt) never waits
                on the previous tile's accumulator-read round trip.
                next_chunks (next batch's phase A) interleave at iterations
                1..8."""
                osb_tiles = {}
                sums = {}
                pending = None

                def finish(j, t_j):
                    ism = small_pool.tile([128, 1], F32, tag="ism", name="ism")
                    nc.vector.reciprocal(ism, sums[j])
                    nc.vector.tensor_scalar_mul(
                        osb_tiles[j // PAIR][:, ts(j % PAIR, M)], t_j, ism
                    )
                    if j % PAIR == PAIR - 1:
                        osb = osb_tiles.pop(j // PAIR)
                        if j == MT - 1:
                            for jj in range(PAIR):
                                nc.sync.dma_start(
                                    out=out_r[b][:, j - PAIR + 1 + jj : j - PAIR + 2 + jj, :],
                                    in_=osb[:, ts(jj, M)],
                                )
                        else:
                            nc.sync.dma_start(
                                out=out_r[b][:, j - PAIR + 1 : j + 1, :],
                                in_=osb,
                            )

                def emit_mm(rt):
                    ps = psum_pool.tile([128, M], F32, tag="ps", name="ps_s")
                    for fc in range(FC):
                        nc.tensor.matmul(
                            ps[:, ts(fc, 512)],
                            lhsT=qkt["q"][:, ts(rt, 128)],
                            rhs=qkt["k"][:, ts(fc, 512)],
                            start=True,
                            stop=True,
                        )
                    return ps

                # Matmuls are emitted one iteration ahead: the next tile's
                # PSUM slot is claimed before any interleaved chunk's, so S
                # matmuls never queue behind a chunk's ACT evacuation.
                ps_next = emit_mm(0)
                for rt in range(MT):
                    ps_s = ps_next
                    # Evacuate S from PSUM to fp16 SBUF with fused row-max
                    # (tensor_scalar accum_out reduces with op1); frees the
                    # PSUM slot so exp reads the SBUF copy instead.
                    sc_t = sc_pool.tile([128, M], FP16, tag="sc", name="sc")
                    mx = small_pool.tile([128, 1], F32, tag="mx", name="mx")
                    if rt == 0:
                        # First tile: evacuate in column halves so the left
                        # half starts before all four matmuls finish and the
                        # reciprocal is ready before tile 1's evac (the
                        # scheduler otherwise runs that 2.2us evac first,
                        # delaying the first exp of each batch).
                        mx2 = small_pool.tile([128, 2], F32, tag="mx2", name="mx2")
                        for hf in range(2):
                            nc.vector.tensor_scalar(
                                sc_t[:, ts(hf, 1024)], ps_s[:, ts(hf, 1024)],
                                0.0, None,
                                op0=ALU.add, op1=ALU.max,
                                accum_out=mx2[:, hf : hf + 1],
                            )
                        nc.vector.tensor_tensor(
                            mx, mx2[:, 0:1], mx2[:, 1:2], op=ALU.max
                        )
                    else:
                        nc.vector.tensor_scalar(
                            sc_t, ps_s, 0.0, None,
                            op0=ALU.add, op1=ALU.max, accum_out=mx,
                        )
                    imx = small_pool.tile([128, 1], F32, tag="imx", name="imx")
                    nc.vector.reciprocal(imx, mx)

                    sums[rt] = small_pool.tile([128, 1], F32, tag="sm", name="sm")
                    t_t = t_pool.tile([128, M], FP16, tag="t")
                    nc.scalar.activation(
                        t_t, sc_t, AF.Exp,
                        bias=0.0, scale=imx, accum_out=sums[rt],
                    )
                    # hoisted matmul emission: after this tile's exp (so the
                    # scheduler orders imx before the next evac) but before
                    # any interleaved chunk claims a PSUM slot
                    ps_next = emit_mm(rt + 1) if rt + 1 < MT else None

                    if rt % PAIR == 0:
                        osb_tiles[rt // PAIR] = out_pool.tile(
                            [128, PAIR * M], FP16, tag="o", name="osb"
                        )
                    if pending is not None:
                        finish(pending[0], pending[1])
                    pending = (rt, t_t)

                    if next_chunks and 1 <= rt <= 8:
                        next_chunks.pop(0)()
                finish(pending[0], pending[1])
                while next_chunks:
                    next_chunks.pop(0)()

            ctx0, chunks0 = phase_a_chunks(0, fast=True)
            for c in chunks0:
                c()
            qkt0 = {"q": ctx0[("qkt", "q")], "k": ctx0[("qkt", "k")]}

            ctx1, chunks1 = phase_a_chunks(1)
            s_loop(0, qkt0, chunks1)
            qkt1 = {"q": ctx1[("qkt", "q")], "k": ctx1[("qkt", "k")]}
            s_loop(1, qkt1, [])
    nc.finalize()
    return nc


_NC_CACHE = None


def _get_nc():
    global _NC_CACHE
    if _NC_CACHE is None:
        _NC_CACHE = build_nc()
    return _NC_CACHE


def run(inputs, trace=False, trace_cores=None):
    """Run on 8 cores; returns (full_output [B,M,M] f32, BassKernelResults)."""
    nc = _get_nc()
    in_maps = []
    x = np.ascontiguousarray(inputs["x"], dtype=np.float32)
    for c in range(N_CORES):
        im = {"x": np.ascontiguousarray(x[c * BPC : (c + 1) * BPC])}
        for k in ("qW1", "qb1", "qW2", "qb2", "kW1", "kb1", "kW2", "kb2"):
            im[k] = np.ascontiguousarray(inputs[k], dtype=np.float32)
        in_maps.append(im)
    res = run_bass_kernel_spmd(
        nc,
        in_maps,
        core_ids=list(range(N_CORES)),
        trace=trace,
        trace_cores=trace_cores,
    )
    outs = [np.asarray(r["out"]) for r in res.results]
    full = np.concatenate(outs, axis=0).astype(np.float32)
    assert full.shape == (B, M, M) and full.dtype == np.float32
    return full, res


def kernel(**inputs) -> np.ndarray:
    out, _ = run(inputs, trace=False)
    return out


# revision 27
# speedup vs baseline: 1.0133x; 1.0133x over previous
"""Self-contained Trainium2 Bass kernel for nn_CA_9363028705415 (sparse_attention).

Computes, per batch b:
    Q = relu(x[b] @ qW1 + qb1) @ qW2 + qb2          # [M, K]
    Kt = relu(x[b] @ kW1 + kb1) @ kW2 + kb2         # [M, K]
    S = Q @ Kt.T                                    # [M, M]
    out[b] = softmax(S / rowmax(S), axis=-1)        # max-DIVISION normalization

Shapes: B=16, M=2048, D=128, H=256, K=64.  Output [16, 2048, 2048] f32 (256 MB).

Sharding: data-parallel over batch across 8 NeuronCores; 2 batches/core; tiny
MLP weights replicated.  Single NEFF run SPMD via run_bass_kernel_spmd.

Design notes (HW-calibrated via perfetto traces):
  - Output written to DRAM as fp16, upcast to f32 on the host: halves the
    dominant HBM write traffic (rows sum to 1, fp16 rel err ~5e-4).
  - x loaded pre-cast f32->bf16 by gpsimd (SWDGE) DMA.
  - Per row-tile pipeline:
      PE   S = Q K^T -> f32 PSUM (TRN2: matmul PSUM output must be f32).
           Matmuls are emitted one iteration ahead so the next tile's PSUM
           slot is claimed before any interleaved chunk's: S matmuls never
           queue behind a chunk's ACT evacuation.
      DVE  fused PSUM->SBUF fp16 evac + row-max accum (~2.1us, frees PSUM)
      DVE  1/max reciprocal -- separate from the 1/sum reciprocal so exp
           never waits on the previous tile's accumulator-read round trip
      ACT  exp(S * (1/max)) fp16->fp16 + fused row-sum accum (~2.25us)
      DVE  1/sum reciprocal + normalize t * (1/sum) fp16->fp16 at DVE 4x
           mode (~0.7us)
      DMA  1 MB fp16 output chunks (2 row-tiles per DMA)
  - DVE is the binding engine in steady state (evac 2.1 + norm 0.7 +
    recips 0.15 ~= 3.0us/tile vs ACT 2.25); measured cadence is 3.00us.
  - Batch-1 phase A (transposes + MLP) interleaves into batch-0's S loop at
    iterations 1..8 (early enough that the batch boundary has no bubble);
    MLP evacs ride ACT (exp/identity/relu/copy share one activation table
    -> no table reloads).
"""

import numpy as np
import ml_dtypes

import concourse.bass as bass
import concourse.mybir as mybir
from concourse import bacc
import concourse.tile as tile
from concourse.bass import ts
from concourse.bass_utils import run_bass_kernel_spmd

F32 = mybir.dt.float32
BF16 = mybir.dt.bfloat16
FP16 = mybir.dt.float16
AF = mybir.ActivationFunctionType
ALU = mybir.AluOpType

N_CORES = 8
B, M, D, H, KF = 16, 2048, 128, 256, 64
BPC = B // N_CORES     # batches per core
MT = M // 128          # 16 row-tiles per batch
FC = M // 512          # 4 matmul free-chunks of 512
PAIR = 2               # row-tiles per output DMA (1 MB fp16 chunks)

RELU_ENGINES = ("act", "act", "act", "act")  # batch-1 MLP1 evac engines
QKT_EVAC = ("act", "act")                    # batch-1 MLP2 evac engines


def _evac_bias(nc, engine, out, in_, bias, relu):
    """out = [relu](in_ + bias), bias is [P,1] per-partition AP."""
    if engine == "act":
        nc.scalar.activation(
            out, in_, AF.Relu if relu else AF.Identity, bias=bias, scale=1.0
        )
    else:
        if relu:
            nc.vector.tensor_scalar(out, in_, bias, 0.0, op0=ALU.add, op1=ALU.max)
        else:
            nc.vector.tensor_scalar(out, in_, bias, None, op0=ALU.add)


def build_nc():
    nc = bacc.Bacc()

    x = nc.dram_tensor("x", [BPC, M, D], F32, kind="ExternalInput")
    w1d, b1d, w2d, b2d = {}, {}, {}, {}
    for h in ("q", "k"):
        w1d[h] = nc.dram_tensor(f"{h}W1", [D, H], F32, kind="ExternalInput")
        b1d[h] = nc.dram_tensor(f"{h}b1", [H], F32, kind="ExternalInput")
        w2d[h] = nc.dram_tensor(f"{h}W2", [H, KF], F32, kind="ExternalInput")
        b2d[h] = nc.dram_tensor(f"{h}b2", [KF], F32, kind="ExternalInput")
    out = nc.dram_tensor("out", [BPC, M, M], FP16, kind="ExternalOutput")

    ident_np = np.eye(128, dtype=ml_dtypes.bfloat16)
    ident_dram = nc.inline_tensor(ident_np, name="ident_data")

    # [b, p, n, d]: token (n*128+p), feature d
    x_r = x[:].rearrange("b (n p) d -> b p n d", p=128)
    # [b, p, n, m]: out[b, n*128+p, m]
    out_r = out[:].rearrange("b (n p) m -> b p n m", p=128)

    with tile.TileContext(nc) as tc:
        with (
            tc.tile_pool(name="consts", bufs=1) as consts,
            tc.tile_pool(name="xin", bufs=2) as xin_pool,
            tc.tile_pool(name="xt", bufs=2) as xt_pool,
            tc.tile_pool(name="ht", bufs=2) as ht_pool,
            tc.tile_pool(name="qkt", bufs=2) as qkt_pool,
            tc.tile_pool(name="sc", bufs=3) as sc_pool,
            tc.tile_pool(name="texp", bufs=3) as t_pool,
            tc.tile_pool(name="osb", bufs=3) as out_pool,
            tc.tile_pool(name="small", bufs=6) as small_pool,
            tc.tile_pool(name="psum", bufs=2, space="PSUM") as psum_pool,
        ):
            # ---- x loads for batch 0 first (gpsimd DMA casts f32->bf16) ----
            ident = consts.tile([128, 128], BF16, tag="ident")
            nc.sync.dma_start(out=ident, in_=ident_dram[:])
            xf = {}
            for b in range(BPC):
                xf[b] = xin_pool.tile([128, MT, 128], BF16, tag=f"xf{b}", name="xf")
            for g in range(4):
                nc.gpsimd.dma_start(
                    out=xf[0][:, g * 4 : (g + 1) * 4, :],
                    in_=x_r[0][:, g * 4 : (g + 1) * 4, :],
                )

            # ---- PE p-state warmup: dense dummy transposes on the identity
            # while waiting for x (cold PE runs matmuls 2-2.5x slower; this
            # burst runs in the 8-11us x-DMA window and lifts the clock
            # before the real transposes arrive) ----
            warm = psum_pool.tile([128, 1024], BF16, tag="ps", name="warm")
            for i in range(24):
                nc.tensor.transpose(warm[:, ts(i % 8, 128)], ident, ident)

            # ---- constants (weights cast f32->bf16 by gpsimd DMA) ----
            w1, w2, b1, b2 = {}, {}, {}, {}
            for h in ("q", "k"):
                w1[h] = consts.tile([D, H], BF16, tag=f"w1{h}", name=f"w1{h}")
                nc.gpsimd.dma_start(out=w1[h], in_=w1d[h][:])  # cast f32->bf16
                w2[h] = consts.tile([128, 2, KF], BF16, tag=f"w2{h}", name=f"w2{h}")
                nc.gpsimd.dma_start(
                    out=w2[h], in_=w2d[h][:].rearrange("(c p) k -> p c k", p=128)
                )
                b1[h] = consts.tile([128, 2], F32, tag=f"b1{h}", name=f"b1{h}")
                nc.sync.dma_start(
                    out=b1[h], in_=b1d[h][:].rearrange("(c p) -> p c", p=128)
                )
                b2[h] = consts.tile([KF, 1], F32, tag=f"b2{h}", name=f"b2{h}")
                nc.sync.dma_start(
                    out=b2[h], in_=b2d[h][:].rearrange("(k o) -> k o", o=1)
                )
            for g in range(4):
                nc.gpsimd.dma_start(
                    out=xf[1][:, g * 4 : (g + 1) * 4, :],
                    in_=x_r[1][:, g * 4 : (g + 1) * 4, :],
                )

            def phase_a_chunks(b, fast=False):
                """Phase-A emission chunks for batch b.  fast=True (batch 0)
                splits evacs across both engines to shorten the serial ramp;
                otherwise engines follow the tuned patterns."""
                ctx = {}
                ctx["xT"] = xt_pool.tile([128, M], BF16, tag="xt", name="xT")

                def c_tp(g):
                    def go():
                        tp = psum_pool.tile([128, 1024], BF16, tag="ps", name="tp")
                        for it in range(8):
                            nc.tensor.transpose(
                                tp[:, ts(it, 128)], xf[b][:, g * 8 + it, :], ident
                            )
                        if fast:
                            nc.scalar.copy(
                                ctx["xT"][:, g * 1024 : g * 1024 + 512], tp[:, 0:512]
                            )
                            nc.vector.tensor_scalar(
                                ctx["xT"][:, g * 1024 + 512 : (g + 1) * 1024],
                                tp[:, 512:1024], 0.0, None, op0=ALU.add,
                            )
                        else:
                            nc.vector.tensor_scalar(
                                ctx["xT"][:, ts(g, 1024)], tp, 0.0, None, op0=ALU.add
                            )
                    return go

                def c_mlp1(h, pc, eng):
                    def go():
                        if ("ht", h) not in ctx:
                            ctx[("ht", h)] = ht_pool.tile(
                                [128, 2, M], BF16, tag=f"ht{h}", name=f"ht{h}"
                            )
                        ps1 = psum_pool.tile([128, M], F32, tag="ps", name="ps1")
                        for fc in range(FC):
                            nc.tensor.matmul(
                                ps1[:, ts(fc, 512)],
                                lhsT=w1[h][:, ts(pc, 128)],
                                rhs=ctx["xT"][:, ts(fc, 512)],
                                start=True,
                                stop=True,
                            )
                        if fast:
                            for e, hf in (("act", 0), ("dve", 1)):
                                _evac_bias(
                                    nc, e,
                                    ctx[("ht", h)][:, pc, ts(hf, 1024)],
                                    ps1[:, ts(hf, 1024)],
                                    b1[h][:, pc : pc + 1],
                                    relu=True,
                                )
                        else:
                            _evac_bias(
                                nc, eng,
                                ctx[("ht", h)][:, pc, :],
                                ps1,
                                b1[h][:, pc : pc + 1],
                                relu=True,
                            )
                    return go

                def c_mlp2(h, eng):
                    def go():
                        ps2 = psum_pool.tile([KF, M], F32, tag="ps", name="ps2")
                        for fc in range(FC):
                            for kc in range(2):
                                nc.tensor.matmul(
                                    ps2[:, ts(fc, 512)],
                                    lhsT=w2[h][:, kc, :],
                                    rhs=ctx[("ht", h)][:, kc, ts(fc, 512)],
                                    start=(kc == 0),
                                    stop=(kc == 1),
                                )
                        q = qkt_pool.tile([KF, M], BF16, tag=f"qkt{h}", name=f"qkt{h}")
                        ctx[("qkt", h)] = q
                        if fast:
                            # chunked evac so the first S matmuls can start on
                            # column chunk 0 before the rest are evacuated
                            for fc in range(FC):
                                _evac_bias(
                                    nc, ("act", "dve")[fc % 2],
                                    q[:, ts(fc, 512)],
                                    ps2[:, ts(fc, 512)],
                                    b2[h],
                                    relu=False,
                                )
                        else:
                            _evac_bias(nc, eng, q, ps2, b2[h], relu=False)
                    return go

                chunks = [c_tp(0), c_tp(1)]
                for i, (h, pc) in enumerate(
                    [("q", 0), ("k", 0), ("q", 1), ("k", 1)]
                ):
                    chunks.append(c_mlp1(h, pc, RELU_ENGINES[i]))
                chunks.append(c_mlp2("q", QKT_EVAC[0]))
                chunks.append(c_mlp2("k", QKT_EVAC[1]))
                return ctx, chunks

            def s_loop(b, qkt, next_chunks, pre_ps=None, next_qkt=None, handoff=None):
                """Emit the S+softmax loop for batch b.  The 1/max and 1/sum
                reciprocals are separate instructions so exp(rt) never waits
                on the previous tile's accumulator-read round trip.
                next_chunks (next batch's phase A) interleave at iterations
                1..8.  At the last iteration the NEXT batch's first-tile
                matmuls are emitted into the freed PSUM ring slot (handoff)
                so the batch boundary has no matmul latency."""
                osb_tiles = {}
                sums = {}
                pending = None

                def finish(j, t_j):
                    ism = small_pool.tile([128, 1], F32, tag="ism", name="ism")
                    nc.vector.reciprocal(ism, sums[j])
                    nc.vector.tensor_scalar_mul(
                        osb_tiles[j // PAIR][:, ts(j % PAIR, M)], t_j, ism
                    )
                    if j % PAIR == PAIR - 1:
                        osb = osb_tiles.pop(j // PAIR)
                        if j == MT - 1:
                            for jj in range(PAIR):
                                nc.sync.dma_start(
                                    out=out_r[b][:, j - PAIR + 1 + jj : j - PAIR + 2 + jj, :],
                                    in_=osb[:, ts(jj, M)],
                                )
                        else:
                            nc.sync.dma_start(
                                out=out_r[b][:, j - PAIR + 1 : j + 1, :],
                                in_=osb,
                            )

                def emit_mm(qk, rt):
                    ps = psum_pool.tile([128, M], F32, tag="ps", name="ps_s")
                    for fc in range(FC):
                        nc.tensor.matmul(
                            ps[:, ts(fc, 512)],
                            lhsT=qk["q"][:, ts(rt, 128)],
                            rhs=qk["k"][:, ts(fc, 512)],
                            start=True,
                            stop=True,
                        )
                    return ps

                # Matmuls are emitted one iteration ahead: the next tile's
                # PSUM slot is claimed before any interleaved chunk's, so S
                # matmuls never queue behind a chunk's ACT evacuation.
                ps_next = pre_ps if pre_ps is not None else emit_mm(qkt, 0)
                for rt in range(MT):
                    if rt + 1 < MT:
                        nxt = emit_mm(qkt, rt + 1)
                    elif next_qkt is not None:
                        handoff["ps"] = emit_mm(next_qkt(), 0)
                        nxt = None
                    else:
                        nxt = None
                    ps_s, ps_next = ps_next, nxt
                    # Evacuate S from PSUM to fp16 SBUF with fused row-max
                    # (tensor_scalar accum_out reduces with op1); frees the
                    # PSUM slot so exp reads the SBUF copy instead.
                    sc_t = sc_pool.tile([128, M], FP16, tag="sc", name="sc")
                    mx = small_pool.tile([128, 1], F32, tag="mx", name="mx")
                    nc.vector.tensor_scalar(
                        sc_t, ps_s, 0.0, None,
                        op0=ALU.add, op1=ALU.max, accum_out=mx,
                    )
                    imx = small_pool.tile([128, 1], F32, tag="imx", name="imx")
                    nc.vector.reciprocal(imx, mx)

                    sums[rt] = small_pool.tile([128, 1], F32, tag="sm", name="sm")
                    t_t = t_pool.tile([128, M], FP16, tag="t")
                    nc.scalar.activation(
                        t_t, sc_t, AF.Exp,
                        bias=0.0, scale=imx, accum_out=sums[rt],
                    )

                    if rt % PAIR == 0:
                        osb_tiles[rt // PAIR] = out_pool.tile(
                            [128, PAIR * M], FP16, tag="o", name="osb"
                        )
                    if pending is not None:
                        finish(pending[0], pending[1])
                    pending = (rt, t_t)

                    if next_chunks and 1 <= rt <= 8:
                        next_chunks.pop(0)()
                finish(pending[0], pending[1])
                while next_chunks:
                    next_chunks.pop(0)()

            ctx0, chunks0 = phase_a_chunks(0, fast=True)
            for c in chunks0:
                c()
            qkt0 = {"q": ctx0[("qkt", "q")], "k": ctx0[("qkt", "k")]}

            ctx1, chunks1 = phase_a_chunks(1)
            ho = {}
            s_loop(
                0, qkt0, chunks1,
                next_qkt=lambda: {"q": ctx1[("qkt", "q")], "k": ctx1[("qkt", "k")]},
                handoff=ho,
            )
            qkt1 = {"q": ctx1[("qkt", "q")], "k": ctx1[("qkt", "k")]}
            s_loop(1, qkt1, [], pre_ps=ho["ps"])
    nc.finalize()
    return nc


_NC_CACHE = None


def _get_nc():
    global _NC_CACHE
    if _NC_CACHE is None:
        _NC_CACHE = build_nc()
    return _NC_CACHE


def run(inputs, trace=False, trace_cores=None):
    """Run on 8 cores; returns (full_output [B,M,M] f32, BassKernelResults)."""
    nc = _get_nc()
    in_maps = []
    x = np.ascontiguousarray(inputs["x"], dtype=np.float32)
    for c in range(N_CORES):
        im = {"x": np.ascontiguousarray(x[c * BPC : (c + 1) * BPC])}
        for k in ("qW1", "qb1", "qW2", "qb2", "kW1", "kb1", "kW2", "kb2"):
            im[k] = np.ascontiguousarray(inputs[k], dtype=np.float32)
        in_maps.append(im)
    res = run_bass_kernel_spmd(
        nc,
        in_maps,
        core_ids=list(range(N_CORES)),
        trace=trace,
        trace_cores=trace_cores,
    )
    outs = [np.asarray(r["out"]) for r in res.results]
    full = np.concatenate(outs, axis=0).astype(np.float32)
    assert full.shape == (B, M, M) and full.dtype == np.float32
    return full, res


def kernel(**inputs) -> np.ndarray:
    out, _ = run(inputs, trace=False)
    return out
